# revision 3
# baseline (speedup 1.0000x reference)
"""MoE top-2-of-8 layer on 8 TRN2 NeuronCores, expert-parallel.

Sharding: expert-parallel — core e owns expert e's weights (bf16, resident in
SBUF). The gate (tiny: 0.01% of FLOPs) runs on host in fp32, tokens are
dispatched to their routed experts' cores (the "all-to-all" happens as part of
host-side sharding), each core runs the dense FFN for its expert over its
gathered token batch as back-to-back bf16 matmuls, and the host combines the
per-expert outputs with the top-2 gate weights.

Device layout is fully transposed so no on-device transposes are needed:
  Ht[f, c] = sum_k w1t[d, f].T @ xt[d, c]      (w1t = w1[e].T, xt = gathered X.T)
  A        = silu(Ht)                           (ScalarE, PSUM -> SBUF bf16)
  Yt[d, c] = sum_f w2t[f, d].T @ A[f, c]        (w2t = w2[e].T)
"""

import numpy as np
import ml_dtypes

import concourse.bass as bass
import concourse.tile as tile
from concourse import mybir
from concourse.bass_utils import run_bass_kernel_spmd

TOP_K = 2
B, S, D, F, E = 4, 2048, 1024, 4096, 8
T = B * S
P = 128
NT = 512  # token chunk (PSUM bank = 512 fp32)

BF16 = mybir.dt.bfloat16
F32 = mybir.dt.float32


def _split_excess_waits(nc):
    """This walrus build accepts at most 1 sync wait per instruction (2 on
    EventSemaphoreOp). Tile can attach more. Hoist the excess onto fresh
    same-engine NOPs spliced immediately before the instruction — the engine
    executes the waits in program order either way, so this is semantically
    identical, just sequential."""
    n_fix = 0
    for bb in nc.m.functions[0].blocks:
        insts = bb.instructions
        if not any(
            getattr(i, "sync_info", None)
            and i.sync_info.on_wait
            and len(i.sync_info.on_wait) > (2 if i.opcode == "EventSemaphoreOp" else 1)
            for i in insts
        ):
            continue
        out = []
        for inst in insts:
            si = getattr(inst, "sync_info", None)
            limit = 2 if inst.opcode == "EventSemaphoreOp" else 1
            if si is not None and si.on_wait and len(si.on_wait) > limit:
                waits = list(si.on_wait)
                si.on_wait[:] = waits[-limit:]
                for w in waits[:-limit]:
                    n_fix += 1
                    nop = mybir.InstNoOp(
                        name=f"I-waitfix-{n_fix}-{inst.name}",
                        engine=inst.engine,
                        ins=[],
                        outs=[],
                        sync_info=mybir.SyncInfo(on_wait=[w], on_update=[]),
                        text_hint="waitfix",
                    )
                    nc.register_instruction(nop, overwrite=True)
                    out.append(nop)
            out.append(inst)
        insts[:] = out


def _patch_tile_drain():
    """Spread the exit drain's accumulated waits over single-wait NOPs and
    run the generic excess-wait splitter over the whole block."""
    if getattr(tile.TileContext, "_drain_patch_installed", False):
        return

    def _drain_and_barrier(self, tick_clock, wait_clock):
        nc = self.nc
        probe = nc.sync.nop(hint="tile_drain_waits")
        wait_clock.add_sem_waits(
            probe.ins, tile.ScopedClock({None: tick_clock.global_clock})
        )
        si = probe.ins.sync_info
        waits = list(si.on_wait) if si is not None else []
        if si is not None:
            si.on_wait[:] = waits[:1]
        for w in waits[1:]:
            n = nc.sync.nop(hint="tile_drain_waits")
            if n.ins.sync_info is None:
                n.ins.sync_info = mybir.SyncInfo(on_wait=[w], on_update=[])
            else:
                n.ins.sync_info.on_wait[:] = [w]
        nc.sync.drain()
        nc.all_engine_barrier()
        assert self.sems is not None
        popped = nc._tile_sem_poison_stack.pop()
        assert popped is self._sem_poison
        nc.clear_and_free_semaphores(list(self.sems.allocated().values()))
        nc.all_engine_barrier()
        _split_excess_waits(nc)

    tile.TileContext._drain_and_barrier = _drain_and_barrier
    tile.TileContext._drain_patch_installed = True


def build_ffn_kernel(C: int) -> bass.Bass:
    """Per-core dense FFN: yt[1024, C] = w2t.T @ silu(w1t.T @ xt) in bf16."""
    assert C % P == 0
    nc = bass.Bass()
    xt = nc.declare_dram_parameter("xt", [D, C], BF16, isOutput=False)
    w1t = nc.declare_dram_parameter("w1t", [D, F], BF16, isOutput=False)
    w2t = nc.declare_dram_parameter("w2t", [F, D], BF16, isOutput=False)
    yt = nc.declare_dram_parameter("yt", [D, C], F32, isOutput=True)

    KD = D // P   # 8 k-tiles for mm1 / d-blocks for mm2
    KF = F // P   # 32 f-blocks for mm1 / k-tiles for mm2

    xt_v = xt.rearrange("(k p) c -> k p c", p=P)
    w1t_v = w1t.rearrange("(k p) f -> k p f", p=P)
    w2t_v = w2t.rearrange("(k p) d -> k p d", p=P)
    yt_v = yt.rearrange("(k p) c -> k p c", p=P)

    chunks = []
    c0 = 0
    while c0 < C:
        nt = min(NT, C - c0)
        chunks.append((c0, nt))
        c0 += nt

    with tile.TileContext(nc) as tc:
        with (
            tc.tile_pool(name="w1p", bufs=KD) as w1p,
            tc.tile_pool(name="w2p", bufs=KF) as w2p,
            tc.tile_pool(name="xp", bufs=2 * KD) as xp,
            tc.tile_pool(name="ap", bufs=KF) as ap_pool,
            tc.tile_pool(name="yp", bufs=KD) as yp,
            tc.tile_pool(name="ph", bufs=3, space="PSUM") as php,
            tc.tile_pool(name="py", bufs=3, space="PSUM") as pyp,
        ):
            w1sb = []
            for k in range(KD):
                t = w1p.tile([P, F], BF16, tag="w1")
                nc.sync.dma_start(t[:], w1t_v[k])
                w1sb.append(t)
            w2sb = []
            for k in range(KF):
                t = w2p.tile([P, D], BF16, tag="w2")
                nc.sync.dma_start(t[:], w2t_v[k])
                w2sb.append(t)

            for c0, nt in chunks:
                xsb = []
                for k in range(KD):
                    t = xp.tile([P, NT], BF16, tag="x")
                    nc.sync.dma_start(t[:, :nt], xt_v[k][:, c0 : c0 + nt])
                    xsb.append(t)

                asb = []
                for f in range(KF):
                    ph = php.tile([P, NT], F32, tag="ph")
                    for k in range(KD):
                        nc.tensor.matmul(
                            ph[:, :nt],
                            lhsT=w1sb[k][:, f * P : (f + 1) * P],
                            rhs=xsb[k][:, :nt],
                            start=(k == 0),
                            stop=(k == KD - 1),
                        )
                    a = ap_pool.tile([P, NT], BF16, tag="a")
                    nc.scalar.activation(
                        a[:, :nt], ph[:, :nt], mybir.ActivationFunctionType.Silu
                    )
                    asb.append(a)

                for d in range(KD):
                    py = pyp.tile([P, NT], F32, tag="py")
                    for f in range(KF):
                        nc.tensor.matmul(
                            py[:, :nt],
                            lhsT=w2sb[f][:, d * P : (d + 1) * P],
                            rhs=asb[f][:, :nt],
                            start=(f == 0),
                            stop=(f == KF - 1),
                        )
                    y = yp.tile([P, NT], F32, tag="y")
                    nc.vector.tensor_copy(y[:, :nt], py[:, :nt])
                    nc.sync.dma_start(yt_v[d][:, c0 : c0 + nt], y[:, :nt])
    return nc


def _route_host(xf: np.ndarray, gate_w: np.ndarray):
    """fp32 gate + top-2 on host. Returns per-expert index lists and weights."""
    logits = xf @ gate_w.T  # [T, E] fp32
    order = np.argsort(-logits, axis=1, kind="stable")
    i1, i2 = order[:, 0], order[:, 1]
    l1 = logits[np.arange(T), i1]
    l2 = logits[np.arange(T), i2]
    # top-2 softmax renormalized == sigmoid of the logit gap
    g1 = 1.0 / (1.0 + np.exp(-(l1 - l2).astype(np.float64)))
    g1 = g1.astype(np.float32)
    g2 = (1.0 - g1).astype(np.float32)
    idx_e, w_e = [], []
    for e in range(E):
        m1 = i1 == e
        m2 = i2 == e
        idx = np.concatenate([np.nonzero(m1)[0], np.nonzero(m2)[0]])
        w = np.concatenate([g1[m1], g2[m2]])
        idx_e.append(idx.astype(np.int64))
        w_e.append(w)
    return idx_e, w_e


def kernel(x, gate_w, w1, w2):
    _patch_tile_drain()
    xf = np.ascontiguousarray(x.reshape(T, D), dtype=np.float32)
    idx_e, w_e = _route_host(xf, np.asarray(gate_w, dtype=np.float32))

    cmax = max(len(i) for i in idx_e)
    C = max(P, ((cmax + P - 1) // P) * P)

    xf_bf = xf.astype(ml_dtypes.bfloat16)
    in_maps = []
    for e in range(E):
        idx = idx_e[e]
        xe = np.zeros((C, D), dtype=ml_dtypes.bfloat16)
        xe[: len(idx)] = xf_bf[idx]
        in_maps.append(
            {
                "xt": np.ascontiguousarray(xe.T),
                "w1t": np.ascontiguousarray(
                    np.asarray(w1[e], dtype=np.float32).astype(ml_dtypes.bfloat16).T
                ),
                "w2t": np.ascontiguousarray(
                    np.asarray(w2[e], dtype=np.float32).astype(ml_dtypes.bfloat16).T
                ),
            }
        )

    nc = build_ffn_kernel(C)
    res = run_bass_kernel_spmd(nc, in_maps, list(range(E)))

    out = np.zeros((T, D), dtype=np.float32)
    for e in range(E):
        idx = idx_e[e]
        yt = res.results[e]["yt"]  # [D, C] fp32
        out[idx] += w_e[e][:, None] * yt.T[: len(idx)]
    return out.reshape(B, S, D)


# revision 4
# speedup vs baseline: 1.0715x; 1.0715x over previous
"""MoE top-2-of-8 layer on 8 TRN2 NeuronCores, expert-parallel.

Sharding: expert-parallel — core e owns expert e's weights (bf16, resident in
SBUF). The gate (tiny: 0.01% of FLOPs) runs on host in fp32, tokens are
dispatched to their routed experts' cores (the "all-to-all" happens as part of
host-side sharding), each core runs the dense FFN for its expert over its
gathered token batch as back-to-back bf16 matmuls, and the host combines the
per-expert outputs with the top-2 gate weights.

Device layout is fully transposed so no on-device transposes are needed:
  Ht[f, c] = sum_k w1t[d, f].T @ xt[d, c]      (w1t = w1[e].T, xt = gathered X.T)
  A        = silu(Ht)                           (ScalarE, PSUM -> SBUF bf16)
  Yt[d, c] = sum_f w2t[f, d].T @ A[f, c]        (w2t = w2[e].T)
"""

import numpy as np
import ml_dtypes

import concourse.bass as bass
import concourse.tile as tile
from concourse import mybir
from concourse.bass_utils import run_bass_kernel_spmd

TOP_K = 2
B, S, D, F, E = 4, 2048, 1024, 4096, 8
T = B * S
P = 128
NT = 512  # token chunk (PSUM bank = 512 fp32)

BF16 = mybir.dt.bfloat16
F32 = mybir.dt.float32


def _split_excess_waits(nc):
    """This walrus build accepts at most 1 sync wait per instruction (2 on
    EventSemaphoreOp). Tile can attach more. Hoist the excess onto fresh
    same-engine NOPs spliced immediately before the instruction — the engine
    executes the waits in program order either way, so this is semantically
    identical, just sequential."""
    n_fix = 0
    for bb in nc.m.functions[0].blocks:
        insts = bb.instructions
        if not any(
            getattr(i, "sync_info", None)
            and i.sync_info.on_wait
            and len(i.sync_info.on_wait) > (2 if i.opcode == "EventSemaphoreOp" else 1)
            for i in insts
        ):
            continue
        out = []
        for inst in insts:
            si = getattr(inst, "sync_info", None)
            limit = 2 if inst.opcode == "EventSemaphoreOp" else 1
            if si is not None and si.on_wait and len(si.on_wait) > limit:
                waits = list(si.on_wait)
                si.on_wait[:] = waits[-limit:]
                for w in waits[:-limit]:
                    n_fix += 1
                    nop = mybir.InstNoOp(
                        name=f"I-waitfix-{n_fix}-{inst.name}",
                        engine=inst.engine,
                        ins=[],
                        outs=[],
                        sync_info=mybir.SyncInfo(on_wait=[w], on_update=[]),
                        text_hint="waitfix",
                    )
                    nc.register_instruction(nop, overwrite=True)
                    out.append(nop)
            out.append(inst)
        insts[:] = out


def _patch_tile_drain():
    """Spread the exit drain's accumulated waits over single-wait NOPs and
    run the generic excess-wait splitter over the whole block."""
    if getattr(tile.TileContext, "_drain_patch_installed", False):
        return

    def _drain_and_barrier(self, tick_clock, wait_clock):
        nc = self.nc
        probe = nc.sync.nop(hint="tile_drain_waits")
        wait_clock.add_sem_waits(
            probe.ins, tile.ScopedClock({None: tick_clock.global_clock})
        )
        si = probe.ins.sync_info
        waits = list(si.on_wait) if si is not None else []
        if si is not None:
            si.on_wait[:] = waits[:1]
        for w in waits[1:]:
            n = nc.sync.nop(hint="tile_drain_waits")
            if n.ins.sync_info is None:
                n.ins.sync_info = mybir.SyncInfo(on_wait=[w], on_update=[])
            else:
                n.ins.sync_info.on_wait[:] = [w]
        nc.sync.drain()
        nc.all_engine_barrier()
        assert self.sems is not None
        popped = nc._tile_sem_poison_stack.pop()
        assert popped is self._sem_poison
        nc.clear_and_free_semaphores(list(self.sems.allocated().values()))
        nc.all_engine_barrier()
        _split_excess_waits(nc)

    tile.TileContext._drain_and_barrier = _drain_and_barrier
    tile.TileContext._drain_patch_installed = True


def build_ffn_kernel(C: int) -> bass.Bass:
    """Per-core dense FFN: yt[1024, C] = w2t.T @ silu(w1t.T @ xt) in bf16."""
    assert C % P == 0
    nc = bass.Bass()
    xt = nc.declare_dram_parameter("xt", [D, C], BF16, isOutput=False)
    w1t = nc.declare_dram_parameter("w1t", [D, F], BF16, isOutput=False)
    w2t = nc.declare_dram_parameter("w2t", [F, D], BF16, isOutput=False)
    yt = nc.declare_dram_parameter("yt", [D, C], F32, isOutput=True)

    KD = D // P   # 8 k-tiles for mm1 / d-blocks for mm2
    KF = F // P   # 32 f-blocks for mm1 / k-tiles for mm2
    WPC = 512     # w1 piece width (columns) for intro streaming
    NPIECE = F // WPC
    FPP = WPC // P  # f-blocks per w1 piece

    xt_v = xt.rearrange("(k p) c -> k p c", p=P)
    w1t_v = w1t.rearrange("(k p) f -> k p f", p=P)
    w2t_v = w2t.rearrange("(k p) d -> k p d", p=P)
    yt_v = yt.rearrange("(k p) c -> k p c", p=P)

    chunks = []
    c0 = 0
    while c0 < C:
        nt = min(NT, C - c0)
        chunks.append((c0, nt))
        c0 += nt

    psum_bufs = 2 if NT > 512 else 3

    with tile.TileContext(nc) as tc:
        with (
            tc.tile_pool(name="w1p", bufs=KD * NPIECE) as w1p,
            tc.tile_pool(name="w2p", bufs=KF) as w2p,
            tc.tile_pool(name="xp", bufs=2 * KD) as xp,
            tc.tile_pool(name="ap", bufs=KF) as ap_pool,
            tc.tile_pool(name="yp", bufs=KD) as yp,
            tc.tile_pool(name="ph", bufs=psum_bufs, space="PSUM") as php,
            tc.tile_pool(name="py", bufs=psum_bufs, space="PSUM") as pyp,
        ):
            # DMA order on the single HW queue is the intro critical path:
            # x chunk 0 first, then w1 in column-piece-major order so mm1 can
            # start after ~1 piece-column and the queue stays ahead of the PE.
            xsb0 = []
            for k in range(KD):
                t = xp.tile([P, NT], BF16, tag="x")
                nt0 = chunks[0][1]
                nc.sync.dma_start(t[:, :nt0], xt_v[k][:, 0:nt0])
                xsb0.append(t)

            w1sb = [[None] * NPIECE for _ in range(KD)]  # [k][piece] -> [P, WPC]
            for pce in range(NPIECE):
                for k in range(KD):
                    t = w1p.tile([P, WPC], BF16, tag="w1")
                    nc.sync.dma_start(t[:], w1t_v[k][:, pce * WPC : (pce + 1) * WPC])
                    w1sb[k][pce] = t
            w2sb = []
            for k in range(KF):
                t = w2p.tile([P, D], BF16, tag="w2")
                nc.sync.dma_start(t[:], w2t_v[k])
                w2sb.append(t)

            for ci, (c0, nt) in enumerate(chunks):
                if ci == 0:
                    xsb = xsb0
                else:
                    xsb = []
                    for k in range(KD):
                        t = xp.tile([P, NT], BF16, tag="x")
                        nc.sync.dma_start(t[:, :nt], xt_v[k][:, c0 : c0 + nt])
                        xsb.append(t)

                asb = []
                for f in range(KF):
                    ph = php.tile([P, NT], F32, tag="ph")
                    pce, fo = divmod(f, FPP)
                    for k in range(KD):
                        nc.tensor.matmul(
                            ph[:, :nt],
                            lhsT=w1sb[k][pce][:, fo * P : (fo + 1) * P],
                            rhs=xsb[k][:, :nt],
                            start=(k == 0),
                            stop=(k == KD - 1),
                        )
                    a = ap_pool.tile([P, NT], BF16, tag="a")
                    nc.scalar.activation(
                        a[:, :nt], ph[:, :nt], mybir.ActivationFunctionType.Silu
                    )
                    asb.append(a)

                for d in range(KD):
                    py = pyp.tile([P, NT], F32, tag="py")
                    for f in range(KF):
                        nc.tensor.matmul(
                            py[:, :nt],
                            lhsT=w2sb[f][:, d * P : (d + 1) * P],
                            rhs=asb[f][:, :nt],
                            start=(f == 0),
                            stop=(f == KF - 1),
                        )
                    y = yp.tile([P, NT], F32, tag="y")
                    nc.vector.tensor_copy(y[:, :nt], py[:, :nt])
                    nc.sync.dma_start(yt_v[d][:, c0 : c0 + nt], y[:, :nt])
    return nc


def _route_host(xf: np.ndarray, gate_w: np.ndarray):
    """fp32 gate + top-2 on host. Returns per-expert index lists and weights."""
    logits = xf @ gate_w.T  # [T, E] fp32
    order = np.argsort(-logits, axis=1, kind="stable")
    i1, i2 = order[:, 0], order[:, 1]
    l1 = logits[np.arange(T), i1]
    l2 = logits[np.arange(T), i2]
    # top-2 softmax renormalized == sigmoid of the logit gap
    g1 = 1.0 / (1.0 + np.exp(-(l1 - l2).astype(np.float64)))
    g1 = g1.astype(np.float32)
    g2 = (1.0 - g1).astype(np.float32)
    idx_e, w_e = [], []
    for e in range(E):
        m1 = i1 == e
        m2 = i2 == e
        idx = np.concatenate([np.nonzero(m1)[0], np.nonzero(m2)[0]])
        w = np.concatenate([g1[m1], g2[m2]])
        idx_e.append(idx.astype(np.int64))
        w_e.append(w)
    return idx_e, w_e


def kernel(x, gate_w, w1, w2):
    _patch_tile_drain()
    xf = np.ascontiguousarray(x.reshape(T, D), dtype=np.float32)
    idx_e, w_e = _route_host(xf, np.asarray(gate_w, dtype=np.float32))

    cmax = max(len(i) for i in idx_e)
    C = max(P, ((cmax + P - 1) // P) * P)

    xf_bf = xf.astype(ml_dtypes.bfloat16)
    in_maps = []
    for e in range(E):
        idx = idx_e[e]
        xe = np.zeros((C, D), dtype=ml_dtypes.bfloat16)
        xe[: len(idx)] = xf_bf[idx]
        in_maps.append(
            {
                "xt": np.ascontiguousarray(xe.T),
                "w1t": np.ascontiguousarray(
                    np.asarray(w1[e], dtype=np.float32).astype(ml_dtypes.bfloat16).T
                ),
                "w2t": np.ascontiguousarray(
                    np.asarray(w2[e], dtype=np.float32).astype(ml_dtypes.bfloat16).T
                ),
            }
        )

    nc = build_ffn_kernel(C)
    res = run_bass_kernel_spmd(nc, in_maps, list(range(E)))

    out = np.zeros((T, D), dtype=np.float32)
    for e in range(E):
        idx = idx_e[e]
        yt = res.results[e]["yt"]  # [D, C] fp32
        out[idx] += w_e[e][:, None] * yt.T[: len(idx)]
    return out.reshape(B, S, D)
